# revision 27
# baseline (speedup 1.0000x reference)
"""Causal single-head attention (B=8, S=2048, E=768, H=64) on 8 TRN2 NeuronCores.

Sharding: data-parallel over batch — one batch element per core, no collectives.

v8: single unified PSUM pool for projection and attention (no phase
barrier) — QKV projection s-blocks interleave with attention tiles, all
input DMAs prefetched up front across both HWDGE queues. Ones-column V
augmentation gives row-sums from the AV matmul; one batched epilogue.
"""

import numpy as np
from contextlib import ExitStack

import concourse.bass as bass
import concourse.tile as tile
from concourse import bacc, mybir
from concourse.bass_utils import run_bass_kernel_spmd

F32 = mybir.dt.float32
F16 = mybir.dt.float16

B, S, E, H = 8, 2048, 768, 64
HA = H + 1             # V augmented with a ones column -> row sums
EC = E // 128          # 6 e-chunks
QT_TILES = S // 128    # 16 query tiles
NEG = -1.0e9
STAG = 2               # AV lags scores by this many tiles


def build_attention_core():
    nc = bacc.Bacc(None, target_bir_lowering=False)
    xt = nc.declare_dram_parameter("xt", (E, S), F16, isOutput=False)
    wqk = nc.declare_dram_parameter("wqk", (E, 128), F16, isOutput=False)
    wv = nc.declare_dram_parameter("wv", (E, H), F16, isOutput=False)
    mask = nc.declare_dram_parameter("mask", (128, 128), F32, isOutput=False)
    ident = nc.declare_dram_parameter("ident", (64, 64), F16, isOutput=False)
    out = nc.declare_dram_parameter("out", (S, H), F32, isOutput=True)

    with ExitStack() as ctx:
        tc = ctx.enter_context(tile.TileContext(nc))
        singles = ctx.enter_context(tc.tile_pool(name="singles", bufs=1))
        sP = ctx.enter_context(tc.tile_pool(name="sP", bufs=4, space="PSUM"))
        prP = ctx.enter_context(tc.tile_pool(name="prP", bufs=1, space="PSUM"))
        oP = ctx.enter_context(tc.tile_pool(name="oP", bufs=1, space="PSUM"))
        pPool = ctx.enter_context(tc.tile_pool(name="pPool", bufs=STAG + 1))
        ptPool = ctx.enter_context(tc.tile_pool(name="ptPool", bufs=STAG + 1))
        stats = ctx.enter_context(tc.tile_pool(name="stats", bufs=2 * (STAG + 2)))

        # ---- constant + input loads, all prefetched up front ----
        # all plain DMAs on SWDGE: HWDGE copies racing in-flight xbar
        # transposes corrupt data, so the HWDGE queues carry only transposes
        wqk_sb = singles.tile([128, EC, 128], F16)
        wv_sb = singles.tile([128, EC, H], F16)
        nc.gpsimd.dma_start(
            out=wqk_sb[:], in_=wqk.rearrange("(c p) m -> p c m", p=128))
        nc.gpsimd.dma_start(
            out=wv_sb[:], in_=wv.rearrange("(c p) m -> p c m", p=128))
        mask_sb = singles.tile([128, 128], F32)
        nc.gpsimd.dma_start(out=mask_sb[:], in_=mask[:])
        id_sb = singles.tile([64, 64], F16)
        nc.gpsimd.dma_start(out=id_sb[:], in_=ident[:])

        xt_sb = singles.tile([128, EC, S], F16)
        for c in range(EC):
            nc.gpsimd.dma_start(
                out=xt_sb[:, c, :], in_=xt[c * 128:(c + 1) * 128, :],
            )

        qt_sb = singles.tile([64, S], F16)
        kt_sb = singles.tile([64, S], F16)
        vt_sb = singles.tile([64, S], F16)
        v_sb = singles.tile([128, QT_TILES, HA], F16)
        nc.vector.memset(v_sb[:, :, H:HA], 1.0)

        # all 16 AV accumulators in one persistent PSUM region
        o_all = oP.tile([128, QT_TILES, HA], F32)

        def emit_proj(sb):
            """QKV projection for one 512-col s-block."""
            cols = bass.ts(sb, 512)
            qk_ps = prP.tile([128, 512], F32, tag="qk")
            with tc.tile_critical():
                for c in range(EC):
                    nc.tensor.matmul(
                        qk_ps[:], lhsT=wqk_sb[:, c, :], rhs=xt_sb[:, c, cols],
                        start=(c == 0), stop=(c == EC - 1),
                    )
            nc.scalar.copy(qt_sb[:, cols], qk_ps[0:64, :])
            nc.scalar.copy(kt_sb[:, cols], qk_ps[64:128, :])
            vt_t = prP.tile([128, 512], F32, tag="qk")
            vt_ps = vt_t[0:64, :]
            with tc.tile_critical():
                for c in range(EC):
                    nc.tensor.matmul(
                        vt_ps, lhsT=wv_sb[:, c, :], rhs=xt_sb[:, c, cols],
                        start=(c == 0), stop=(c == EC - 1),
                    )
            nc.scalar.copy(vt_sb[:, cols], vt_ps)
            for j in range(sb * 4, sb * 4 + 4):
                vtr_t = prP.tile([128, 512], F32, tag="qk")
                vtr = vtr_t[:, 0:32].bitcast(F16)
                nc.tensor.transpose(
                    vtr, vt_sb[:, j * 128:(j + 1) * 128], id_sb[:]
                )
                nc.vector.tensor_copy(v_sb[:, j, 0:H], vtr)

        live = {}

        def emit_front(i):
            """scores + softmax + transpose for tile i"""
            ki = (i + 1) * 128
            nblk = (ki + 511) // 512
            q_sl = bass.ts(i, 128)
            mx = stats.tile([128, 5], F32, tag="mx")
            negm = stats.tile([128, 1], F32, tag="negm")

            s_tiles = []
            for b in range(nblk):
                w = min(512, ki - b * 512)
                s_t = sP.tile([128, 512], F32, tag="s")
                s_tiles.append((s_t, w))
                nc.tensor.matmul(
                    s_t[:, 0:w],
                    lhsT=qt_sb[:, q_sl],
                    rhs=kt_sb[:, b * 512:b * 512 + w],
                    start=True, stop=True,
                )
                if b == nblk - 1:
                    nc.vector.tensor_add(
                        s_t[:, w - 128:w], s_t[:, w - 128:w], mask_sb[:]
                    )
                nc.vector.tensor_reduce(
                    mx[:, b:b + 1], s_t[:, 0:w],
                    axis=mybir.AxisListType.X, op=mybir.AluOpType.max,
                )
            nc.vector.tensor_reduce(
                negm[:], mx[:, 0:nblk],
                axis=mybir.AxisListType.X, op=mybir.AluOpType.max,
                negate=True,
            )

            p_t = pPool.tile([128, S], F16, tag="p")
            for b, (s_t, w) in enumerate(s_tiles):
                nc.scalar.activation(
                    p_t[:, b * 512:b * 512 + w], s_t[:, 0:w],
                    mybir.ActivationFunctionType.Exp,
                    bias=negm[:], scale=1.0,
                )

            pt_t = ptPool.tile([128, QT_TILES, 128], F16, tag="pt")
            nc.sync.dma_start(
                out=pt_t[:, 0:i + 1, :], in_=p_t[:, 0:ki], transpose=True,
            )
            live[i] = pt_t

        def emit_back(i):
            """AV (+fused row-sum) accumulating into o_all[:, i, :]"""
            pt_t = live.pop(i)
            with tc.tile_critical():
                for j in range(i + 1):
                    nc.tensor.matmul(
                        o_all[:, i, :], lhsT=pt_t[:, j, :], rhs=v_sb[:, j, :],
                        start=(j == 0), stop=(j == i),
                    )

        # interleaved emission: projection block sb, then its 4 query
        # tiles; AV trails the front stage by STAG tiles
        seen = []
        for sb in range(4):
            emit_proj(sb)
            for t in range(sb * 4, sb * 4 + 4):
                if len(seen) >= STAG:
                    emit_back(seen[-STAG])
                emit_front(t)
                seen.append(t)
        for t in seen[-STAG:]:
            emit_back(t)

        # batched epilogue: one reciprocal, one broadcast multiply, one store
        rs_all = stats.tile([128, QT_TILES], F32, tag="rsall")
        nc.vector.reciprocal(rs_all[:], o_all[:, :, H])
        o_fin = singles.tile([128, QT_TILES, H], F32)
        rs_ap = rs_all[:]
        rs_bcast = bass.AP(
            tensor=rs_ap.tensor,
            offset=rs_ap.offset,
            ap=[rs_ap.ap[0], rs_ap.ap[1], [0, H]],
        )
        nc.vector.tensor_mul(o_fin[:], o_all[:, :, 0:H], rs_bcast)
        nc.gpsimd.dma_start(
            out=out.rearrange("(i p) h -> p i h", p=128), in_=o_fin[:]
        )

    nc.finalize()
    return nc


_NC_CACHE = None


def make_in_maps(x, Wq, Wk, Wv):
    scale = np.sqrt(np.float32(E))
    wqk_np = np.concatenate([(Wq * scale).T, Wk.T], axis=1).astype(np.float16)
    wv_np = Wv.T.astype(np.float16)
    mask_np = np.triu(np.full((128, 128), NEG, dtype=np.float32), k=1)
    ident_np = np.eye(64, dtype=np.float16)
    return [
        {
            "xt": np.ascontiguousarray(x[b].T).astype(np.float16),
            "wqk": wqk_np,
            "wv": wv_np,
            "mask": mask_np,
            "ident": ident_np,
        }
        for b in range(B)
    ]


def kernel(x: np.ndarray, Wq: np.ndarray, Wk: np.ndarray, Wv: np.ndarray) -> np.ndarray:
    global _NC_CACHE
    assert x.shape == (B, S, E)
    in_maps = make_in_maps(x, Wq, Wk, Wv)

    if _NC_CACHE is None:
        _NC_CACHE = build_attention_core()
    res = run_bass_kernel_spmd(_NC_CACHE, in_maps, core_ids=list(range(B)))
    return np.stack([res.results[b]["out"] for b in range(B)], axis=0)


if __name__ == "__main__":
    rng = np.random.default_rng(0)
    x = rng.standard_normal((B, S, E), dtype=np.float32)
    sc = 1.0 / np.sqrt(E)
    Wq = rng.uniform(-sc, sc, (H, E)).astype(np.float32)
    Wk = rng.uniform(-sc, sc, (H, E)).astype(np.float32)
    Wv = rng.uniform(-sc, sc, (H, E)).astype(np.float32)
    o = kernel(x=x, Wq=Wq, Wk=Wk, Wv=Wv)
    print(o.shape, o.dtype)


# revision 29
# speedup vs baseline: 1.5462x; 1.5462x over previous
"""Causal single-head attention (B=8, S=2048, E=768, H=64) on 8 TRN2 NeuronCores.

Sharding: data-parallel over batch — one batch element per core, no collectives.

v4: ones-column appended to V so the AV matmul computes row-sums for free
(no accum_out chain); 512-col PSUM score slots (6 bufs) for deeper cross-
tile pipelining; input DMAs split across both HWDGE queues; 2-tile stagger.
"""

import numpy as np
from contextlib import ExitStack

import concourse.bass as bass
import concourse.tile as tile
from concourse import bacc, mybir
from concourse.bass_utils import run_bass_kernel_spmd

F32 = mybir.dt.float32
F16 = mybir.dt.float16

B, S, E, H = 8, 2048, 768, 64
HA = H + 1             # V augmented with a ones column -> row sums
EC = E // 128          # 6 e-chunks
QT_TILES = S // 128    # 16 query tiles
NEG = -1.0e9
STAG = 2               # AV lags scores by this many tiles


def build_attention_core():
    nc = bacc.Bacc(None, target_bir_lowering=False)
    xt = nc.declare_dram_parameter("xt", (E, S), F16, isOutput=False)
    wqk = nc.declare_dram_parameter("wqk", (E, 128), F16, isOutput=False)
    wv = nc.declare_dram_parameter("wv", (E, H), F16, isOutput=False)
    mask = nc.declare_dram_parameter("mask", (128, 128), F32, isOutput=False)
    ident = nc.declare_dram_parameter("ident", (64, 64), F16, isOutput=False)
    out = nc.declare_dram_parameter("out", (S, H), F32, isOutput=True)

    with ExitStack() as ctx:
        tc = ctx.enter_context(tile.TileContext(nc))
        singles = ctx.enter_context(tc.tile_pool(name="singles", bufs=1))

        # ---- constant loads (sync queue; xt split across both HWDGE queues)
        wqk_sb = singles.tile([128, EC, 128], F16)
        wv_sb = singles.tile([128, EC, H], F16)
        for c in range(EC):
            nc.sync.dma_start(out=wqk_sb[:, c, :], in_=wqk[c * 128:(c + 1) * 128, :])
            nc.sync.dma_start(out=wv_sb[:, c, :], in_=wv[c * 128:(c + 1) * 128, :])
        mask_sb = singles.tile([128, 128], F32)
        nc.sync.dma_start(out=mask_sb[:], in_=mask[:])
        id_sb = singles.tile([64, 64], F16)
        nc.sync.dma_start(out=id_sb[:], in_=ident[:])

        xt_sb = singles.tile([128, EC, S], F16)
        qt_sb = singles.tile([64, S], F16)
        kt_sb = singles.tile([64, S], F16)
        vt_sb = singles.tile([64, S], F16)
        v_sb = singles.tile([128, QT_TILES, HA], F16)
        # ones column for all key tiles (row-sum trick)
        nc.vector.memset(v_sb[:, :, H:HA], 1.0)

        # ---- Phase A: QKV projection, streamed by 512-col s-block ----
        with (
            tc.tile_pool(name="psA", bufs=1, space="PSUM") as psA,
            tc.tile_pool(name="psV", bufs=2, space="PSUM") as psV,
            tc.tile_pool(name="psVT", bufs=2, space="PSUM") as psVT,
        ):
            qk_ps = psA.tile([128, S], F32)
            for sb in range(4):
                cols = bass.ts(sb, 512)
                for c in range(EC):
                    eng = nc.scalar if c % 2 == 0 else nc.sync
                    eng.dma_start(
                        out=xt_sb[:, c, cols],
                        in_=xt[c * 128:(c + 1) * 128, sb * 512:(sb + 1) * 512],
                    )
                for c in range(EC):
                    nc.tensor.matmul(
                        qk_ps[:, cols], lhsT=wqk_sb[:, c, :],
                        rhs=xt_sb[:, c, cols],
                        start=(c == 0), stop=(c == EC - 1),
                    )
                vt_ps = psV.tile([64, 512], F32, tag="vt")
                for c in range(EC):
                    nc.tensor.matmul(
                        vt_ps[:], lhsT=wv_sb[:, c, :],
                        rhs=xt_sb[:, c, cols],
                        start=(c == 0), stop=(c == EC - 1),
                    )
                nc.scalar.copy(qt_sb[:, cols], qk_ps[0:64, cols])
                nc.scalar.copy(kt_sb[:, cols], qk_ps[64:128, cols])
                nc.scalar.copy(vt_sb[:, cols], vt_ps[:])
                # V back to [k, h] layout via PE transposes
                for j in range(sb * 4, sb * 4 + 4):
                    vtr = psVT.tile([128, H], F16, tag="vtr")
                    nc.tensor.transpose(
                        vtr[:], vt_sb[:, j * 128:(j + 1) * 128], id_sb[:]
                    )
                    nc.vector.tensor_copy(v_sb[:, j, 0:H], vtr[:])

        # ---- Phase B: software-pipelined attention ----
        with (
            tc.tile_pool(name="sP", bufs=5, space="PSUM") as sP,
            tc.tile_pool(name="oP", bufs=1, space="PSUM") as oP,
            tc.tile_pool(name="pPool", bufs=STAG + 1) as pPool,
            tc.tile_pool(name="ptPool", bufs=STAG + 1) as ptPool,
            tc.tile_pool(name="stats", bufs=2 * (STAG + 2)) as stats,
        ):
            # all 16 AV accumulators live in one persistent PSUM region;
            # normalization happens once at the end
            o_all = oP.tile([128, QT_TILES, HA], F32)
            live = {}

            def emit_front(i):
                """scores + softmax + transpose for tile i"""
                ki = (i + 1) * 128
                nblk = (ki + 511) // 512
                q_sl = bass.ts(i, 128)
                mx = stats.tile([128, 5], F32, tag="mx")
                negm = stats.tile([128, 1], F32, tag="negm")

                s_tiles = []
                n_mx = 0
                for b in range(nblk):
                    w = min(512, ki - b * 512)
                    s_t = sP.tile([128, 512], F32, tag="s")
                    s_tiles.append((s_t, w))
                    nc.tensor.matmul(
                        s_t[:, 0:w],
                        lhsT=qt_sb[:, q_sl],
                        rhs=kt_sb[:, b * 512:b * 512 + w],
                        start=True, stop=True,
                    )
                    if b == nblk - 1:
                        nc.vector.tensor_add(
                            s_t[:, w - 128:w], s_t[:, w - 128:w], mask_sb[:]
                        )
                    nc.vector.tensor_reduce(
                        mx[:, n_mx:n_mx + 1], s_t[:, 0:w],
                        axis=mybir.AxisListType.X, op=mybir.AluOpType.max,
                    )
                    n_mx += 1
                nc.vector.tensor_reduce(
                    negm[:], mx[:, 0:n_mx],
                    axis=mybir.AxisListType.X, op=mybir.AluOpType.max,
                    negate=True,
                )

                p_t = pPool.tile([128, S], F16, tag="p")
                for b, (s_t, w) in enumerate(s_tiles):
                    nc.scalar.activation(
                        p_t[:, b * 512:b * 512 + w], s_t[:, 0:w],
                        mybir.ActivationFunctionType.Exp,
                        bias=negm[:], scale=1.0,
                    )

                pt_t = ptPool.tile([128, QT_TILES, 128], F16, tag="pt")
                nc.sync.dma_start(
                    out=pt_t[:, 0:i + 1, :], in_=p_t[:, 0:ki], transpose=True,
                )
                live[i] = pt_t

            def emit_back(i):
                """AV (+fused row-sum) accumulating into o_all[:, i, :]"""
                pt_t = live.pop(i)
                for j in range(i + 1):
                    nc.tensor.matmul(
                        o_all[:, i, :], lhsT=pt_t[:, j, :], rhs=v_sb[:, j, :],
                        start=(j == 0), stop=(j == i),
                    )

            # back-stage first each iteration: every engine's next
            # instruction depends only on work from >=1 iteration ago
            for t in range(QT_TILES + STAG):
                if t >= STAG:
                    emit_back(t - STAG)
                if t < QT_TILES:
                    emit_front(t)

            # batched epilogue: one reciprocal, one broadcast multiply,
            # one store for all 16 tiles
            rs_all = stats.tile([128, QT_TILES], F32, tag="rsall")
            nc.vector.reciprocal(rs_all[:], o_all[:, :, H])
            o_fin = singles.tile([128, QT_TILES, H], F32)
            rs_ap = rs_all[:]
            rs_bcast = bass.AP(
                tensor=rs_ap.tensor,
                offset=rs_ap.offset,
                ap=[rs_ap.ap[0], rs_ap.ap[1], [0, H]],
            )
            nc.vector.tensor_mul(o_fin[:], o_all[:, :, 0:H], rs_bcast)
            nc.gpsimd.dma_start(
                out=out.rearrange("(i p) h -> p i h", p=128), in_=o_fin[:]
            )

    nc.finalize()
    return nc


_NC_CACHE = None


def make_in_maps(x, Wq, Wk, Wv):
    scale = np.sqrt(np.float32(E))
    wqk_np = np.concatenate([(Wq * scale).T, Wk.T], axis=1).astype(np.float16)
    wv_np = Wv.T.astype(np.float16)
    mask_np = np.triu(np.full((128, 128), NEG, dtype=np.float32), k=1)
    ident_np = np.eye(64, dtype=np.float16)
    return [
        {
            "xt": np.ascontiguousarray(x[b].T).astype(np.float16),
            "wqk": wqk_np,
            "wv": wv_np,
            "mask": mask_np,
            "ident": ident_np,
        }
        for b in range(B)
    ]


def kernel(x: np.ndarray, Wq: np.ndarray, Wk: np.ndarray, Wv: np.ndarray) -> np.ndarray:
    global _NC_CACHE
    assert x.shape == (B, S, E)
    in_maps = make_in_maps(x, Wq, Wk, Wv)

    if _NC_CACHE is None:
        _NC_CACHE = build_attention_core()
    res = run_bass_kernel_spmd(_NC_CACHE, in_maps, core_ids=list(range(B)))
    return np.stack([res.results[b]["out"] for b in range(B)], axis=0)


if __name__ == "__main__":
    rng = np.random.default_rng(0)
    x = rng.standard_normal((B, S, E), dtype=np.float32)
    sc = 1.0 / np.sqrt(E)
    Wq = rng.uniform(-sc, sc, (H, E)).astype(np.float32)
    Wk = rng.uniform(-sc, sc, (H, E)).astype(np.float32)
    Wv = rng.uniform(-sc, sc, (H, E)).astype(np.float32)
    o = kernel(x=x, Wq=Wq, Wk=Wk, Wv=Wv)
    print(o.shape, o.dtype)


# revision 34
# speedup vs baseline: 1.5569x; 1.0069x over previous
"""Causal single-head attention (B=8, S=2048, E=768, H=64) on 8 TRN2 NeuronCores.

Sharding: data-parallel over batch — one batch element per core, no collectives.

v4: ones-column appended to V so the AV matmul computes row-sums for free
(no accum_out chain); 512-col PSUM score slots (6 bufs) for deeper cross-
tile pipelining; input DMAs split across both HWDGE queues; 2-tile stagger.
"""

import numpy as np
from contextlib import ExitStack

import concourse.bass as bass
import concourse.tile as tile
from concourse import bacc, mybir
from concourse.bass_utils import run_bass_kernel_spmd

F32 = mybir.dt.float32
F16 = mybir.dt.float16

B, S, E, H = 8, 2048, 768, 64
HA = H + 1             # V augmented with a ones column -> row sums
EC = E // 128          # 6 e-chunks
QT_TILES = S // 128    # 16 query tiles
NEG = -1.0e9
STAG = 3               # AV lags scores by this many tiles


def build_attention_core():
    nc = bacc.Bacc(None, target_bir_lowering=False)
    xt = nc.declare_dram_parameter("xt", (E, S), F16, isOutput=False)
    wqk = nc.declare_dram_parameter("wqk", (E, 128), F16, isOutput=False)
    wv = nc.declare_dram_parameter("wv", (E, H), F16, isOutput=False)
    mask = nc.declare_dram_parameter("mask", (128, 128), F32, isOutput=False)
    ident = nc.declare_dram_parameter("ident", (64, 64), F16, isOutput=False)
    out = nc.declare_dram_parameter("out", (S, H), F32, isOutput=True)

    with ExitStack() as ctx:
        tc = ctx.enter_context(tile.TileContext(nc))
        singles = ctx.enter_context(tc.tile_pool(name="singles", bufs=1))

        # ---- constant loads (sync queue; xt split across both HWDGE queues)
        wqk_sb = singles.tile([128, EC, 128], F16)
        wv_sb = singles.tile([128, EC, H], F16)
        nc.gpsimd.dma_start(
            out=wqk_sb[:], in_=wqk.rearrange("(c p) m -> p c m", p=128))
        nc.gpsimd.dma_start(
            out=wv_sb[:], in_=wv.rearrange("(c p) m -> p c m", p=128))
        mask_sb = singles.tile([128, 128], F32)
        nc.gpsimd.dma_start(out=mask_sb[:], in_=mask[:])
        id_sb = singles.tile([64, 64], F16)
        nc.gpsimd.dma_start(out=id_sb[:], in_=ident[:])

        xt_sb = singles.tile([128, EC, S], F16)
        for c in range(EC):
            nc.gpsimd.dma_start(out=xt_sb[:, c, :], in_=xt[c * 128:(c + 1) * 128, :])
        qt_sb = singles.tile([64, S], F16)
        kt_sb = singles.tile([64, S], F16)
        vt_sb = singles.tile([64, S], F16)
        v_sb = singles.tile([128, QT_TILES, HA], F16)
        # ones column for all key tiles (row-sum trick)
        nc.vector.memset(v_sb[:, :, H:HA], 1.0)

        # ---- Phase A: QKV projection, streamed by 512-col s-block ----
        with (
            tc.tile_pool(name="psA", bufs=1, space="PSUM") as psA,
            tc.tile_pool(name="psV", bufs=2, space="PSUM") as psV,
            tc.tile_pool(name="psVT", bufs=2, space="PSUM") as psVT,
        ):
            qk_ps = psA.tile([128, S], F32)
            for sb in range(4):
                cols = bass.ts(sb, 512)
                for c in range(EC):
                    nc.tensor.matmul(
                        qk_ps[:, cols], lhsT=wqk_sb[:, c, :],
                        rhs=xt_sb[:, c, cols],
                        start=(c == 0), stop=(c == EC - 1),
                    )
                vt_ps = psV.tile([64, 512], F32, tag="vt")
                for c in range(EC):
                    nc.tensor.matmul(
                        vt_ps[:], lhsT=wv_sb[:, c, :],
                        rhs=xt_sb[:, c, cols],
                        start=(c == 0), stop=(c == EC - 1),
                    )
                nc.scalar.copy(qt_sb[:, cols], qk_ps[0:64, cols])
                nc.scalar.copy(kt_sb[:, cols], qk_ps[64:128, cols])
                nc.scalar.copy(vt_sb[:, cols], vt_ps[:])
                # V back to [k, h] layout via PE transposes
                for j in range(sb * 4, sb * 4 + 4):
                    vtr = psVT.tile([128, H], F16, tag="vtr")
                    nc.tensor.transpose(
                        vtr[:], vt_sb[:, j * 128:(j + 1) * 128], id_sb[:]
                    )
                    nc.vector.tensor_copy(v_sb[:, j, 0:H], vtr[:])

        # ---- Phase B: software-pipelined attention ----
        with (
            tc.tile_pool(name="sP", bufs=5, space="PSUM") as sP,
            tc.tile_pool(name="oP", bufs=1, space="PSUM") as oP,
            tc.tile_pool(name="pPool", bufs=STAG + 1) as pPool,
            tc.tile_pool(name="ptPool", bufs=STAG + 1) as ptPool,
            tc.tile_pool(name="stats", bufs=2 * (STAG + 2)) as stats,
        ):
            # all 16 AV accumulators live in one persistent PSUM region;
            # normalization happens once at the end
            o_all = oP.tile([128, QT_TILES, HA], F32)
            live = {}

            def emit_front(i):
                """scores + softmax + transpose for tile i"""
                ki = (i + 1) * 128
                nblk = (ki + 511) // 512
                q_sl = bass.ts(i, 128)
                mx = stats.tile([128, 5], F32, tag="mx")
                negm = stats.tile([128, 1], F32, tag="negm")

                s_tiles = []
                n_mx = 0
                for b in range(nblk):
                    w = min(512, ki - b * 512)
                    s_t = sP.tile([128, 512], F32, tag="s")
                    s_tiles.append((s_t, w))
                    nc.tensor.matmul(
                        s_t[:, 0:w],
                        lhsT=qt_sb[:, q_sl],
                        rhs=kt_sb[:, b * 512:b * 512 + w],
                        start=True, stop=True,
                    )
                    if b == nblk - 1:
                        nc.vector.tensor_add(
                            s_t[:, w - 128:w], s_t[:, w - 128:w], mask_sb[:]
                        )
                    nc.vector.tensor_reduce(
                        mx[:, n_mx:n_mx + 1], s_t[:, 0:w],
                        axis=mybir.AxisListType.X, op=mybir.AluOpType.max,
                    )
                    n_mx += 1
                nc.vector.tensor_reduce(
                    negm[:], mx[:, 0:n_mx],
                    axis=mybir.AxisListType.X, op=mybir.AluOpType.max,
                    negate=True,
                )

                p_t = pPool.tile([128, S], F16, tag="p")
                for b, (s_t, w) in enumerate(s_tiles):
                    nc.scalar.activation(
                        p_t[:, b * 512:b * 512 + w], s_t[:, 0:w],
                        mybir.ActivationFunctionType.Exp,
                        bias=negm[:], scale=1.0,
                    )

                pt_t = ptPool.tile([128, QT_TILES, 128], F16, tag="pt")
                nc.sync.dma_start(
                    out=pt_t[:, 0:i + 1, :], in_=p_t[:, 0:ki], transpose=True,
                )
                live[i] = pt_t

            def emit_back(i):
                """AV (+fused row-sum) accumulating into o_all[:, i, :]"""
                pt_t = live.pop(i)
                for j in range(i + 1):
                    nc.tensor.matmul(
                        o_all[:, i, :], lhsT=pt_t[:, j, :], rhs=v_sb[:, j, :],
                        start=(j == 0), stop=(j == i),
                    )

            # back-stage first each iteration: every engine's next
            # instruction depends only on work from >=1 iteration ago
            for t in range(QT_TILES + STAG):
                if t >= STAG:
                    emit_back(t - STAG)
                if t < QT_TILES:
                    emit_front(t)

            # batched epilogue: one reciprocal, one broadcast multiply,
            # one store for all 16 tiles
            rs_all = stats.tile([128, QT_TILES], F32, tag="rsall")
            nc.vector.reciprocal(rs_all[:], o_all[:, :, H])
            o_fin = singles.tile([128, QT_TILES, H], F32)
            rs_ap = rs_all[:]
            rs_bcast = bass.AP(
                tensor=rs_ap.tensor,
                offset=rs_ap.offset,
                ap=[rs_ap.ap[0], rs_ap.ap[1], [0, H]],
            )
            nc.vector.tensor_mul(o_fin[:], o_all[:, :, 0:H], rs_bcast)
            nc.gpsimd.dma_start(
                out=out.rearrange("(i p) h -> p i h", p=128), in_=o_fin[:]
            )

    nc.finalize()
    return nc


_NC_CACHE = None


def make_in_maps(x, Wq, Wk, Wv):
    scale = np.sqrt(np.float32(E))
    wqk_np = np.concatenate([(Wq * scale).T, Wk.T], axis=1).astype(np.float16)
    wv_np = Wv.T.astype(np.float16)
    mask_np = np.triu(np.full((128, 128), NEG, dtype=np.float32), k=1)
    ident_np = np.eye(64, dtype=np.float16)
    return [
        {
            "xt": np.ascontiguousarray(x[b].T).astype(np.float16),
            "wqk": wqk_np,
            "wv": wv_np,
            "mask": mask_np,
            "ident": ident_np,
        }
        for b in range(B)
    ]


def kernel(x: np.ndarray, Wq: np.ndarray, Wk: np.ndarray, Wv: np.ndarray) -> np.ndarray:
    global _NC_CACHE
    assert x.shape == (B, S, E)
    in_maps = make_in_maps(x, Wq, Wk, Wv)

    if _NC_CACHE is None:
        _NC_CACHE = build_attention_core()
    res = run_bass_kernel_spmd(_NC_CACHE, in_maps, core_ids=list(range(B)))
    return np.stack([res.results[b]["out"] for b in range(B)], axis=0)


if __name__ == "__main__":
    rng = np.random.default_rng(0)
    x = rng.standard_normal((B, S, E), dtype=np.float32)
    sc = 1.0 / np.sqrt(E)
    Wq = rng.uniform(-sc, sc, (H, E)).astype(np.float32)
    Wk = rng.uniform(-sc, sc, (H, E)).astype(np.float32)
    Wv = rng.uniform(-sc, sc, (H, E)).astype(np.float32)
    o = kernel(x=x, Wq=Wq, Wk=Wk, Wv=Wv)
    print(o.shape, o.dtype)
